# revision 1
# baseline (speedup 1.0000x reference)
"""CQAttention (QANet context-query attention) on 8 Trainium2 NeuronCores.

Full inputs in, full output out. Data-parallel over batch B=32 -> 4 batches
per core. See _build_program() for the per-core Bass/Tile program.

Math notes (vs the jax reference):
  - `bias` and the cross-terms sub0/sub1 that are constant along a softmax
    axis drop out of that softmax; sub1 enters S1's logits as a per-q bias,
    sub0 enters S2's logits via folding w4C into the rhs of the S2 matmul.
  - S1 = softmax_q(sub2 + sub1 + Qmaskbias): computed transposed [q, c];
    row-sum rs[c] over q via an all-ones [q,128] lhsT matmul (result arrives
    pre-broadcast over 128 partitions); 1/rs applied to the final A^T/Bt^T.
  - S2 = softmax_c(sub2 + sub0 + Cmaskbias): computed [c, q]; the c-mask is
    applied multiplicatively (on transposed C rows and on the column-sum
    matmul's lhsT), so exp needs no per-chunk bias.
  - A^T = Qt^T-weighted sums, Bt^T = S1t-weighted V, V = S2^T @ Ct, all via
    PE matmuls with the contraction dim on partitions.
"""

import os
import sys

for _p in ("/opt/trn_rl_repo", "/root/.axon_site/_ro/trn_rl_repo"):
    if os.path.isdir(_p) and _p not in sys.path:
        sys.path.insert(0, _p)

import numpy as np

N_CORES = 8
B_FULL = 32
BPC = B_FULL // N_CORES  # batches per core
D = 128
LC = 2048
LQ = 256
NEG_BIG = -30000.0

_CACHE = {}


def _build_program(repeat=1):
    import os as _os
    O34_ENG = _os.environ.get("O34_ENG", "vector")
    import concourse.mybir as mybir
    import concourse.tile as tile
    from concourse import bacc
    from concourse.masks import make_identity

    f32 = mybir.dt.float32
    f32r = mybir.dt.float32r
    AF = mybir.ActivationFunctionType
    OP = mybir.AluOpType

    nc = bacc.Bacc("TRN2", target_bir_lowering=False, debug=False)

    Cd = nc.dram_tensor("C", [BPC, D, LC], f32r, kind="ExternalInput")
    Qd = nc.dram_tensor("Q", [BPC, D, LQ], f32r, kind="ExternalInput")
    nQd = nc.dram_tensor("negQm", [BPC, D, 2], f32, kind="ExternalInput")
    Cmd = nc.dram_tensor("Cmf", [BPC, D, 16], f32r, kind="ExternalInput")
    wmlud = nc.dram_tensor("wmlu", [D, 1], f32, kind="ExternalInput")
    wcd = nc.dram_tensor("wc", [D, 1], f32, kind="ExternalInput")
    wqd = nc.dram_tensor("wq", [D, 1], f32, kind="ExternalInput")
    outd = nc.dram_tensor("out", [BPC, 3 * D, LC], f32, kind="ExternalOutput")

    def v32(ap):
        return ap.bitcast(f32)

    with tile.TileContext(nc) as tc:
        with (
            tc.tile_pool(name="const", bufs=1) as constp,
            tc.tile_pool(name="big", bufs=2) as sb,
            tc.tile_pool(name="small", bufs=2) as sbs,
            tc.tile_pool(name="psbig", bufs=3, space="PSUM") as psbig,
            tc.tile_pool(name="pssm", bufs=2, space="PSUM") as pssm,
            tc.tile_pool(name="dram", bufs=2, space="DRAM") as dramp,
        ):
            ident32 = constp.tile([128, 128], f32)
            make_identity(nc, ident32[:])
            ident = constp.tile([128, 128], f32r)
            nc.vector.tensor_copy(ident[:], ident32[:])
            ones32 = constp.tile([128, 128], f32)
            nc.vector.memset(ones32[:], 1.0)
            onesm = constp.tile([128, 128], f32r)
            nc.vector.tensor_copy(onesm[:], ones32[:])
            wmlu = constp.tile([D, 1], f32)
            nc.sync.dma_start(out=wmlu[:], in_=wmlud.ap())
            wc = constp.tile([D, 1], f32)
            nc.sync.dma_start(out=wc[:], in_=wcd.ap())
            wq = constp.tile([D, 1], f32)
            nc.sync.dma_start(out=wq[:], in_=wqd.ap())
            csx = constp.tile([128, 256], f32)
            nc.vector.memset(csx[:], 0.0)

            import contextlib
            loop_cm = tc.For_i(0, repeat) if repeat > 1 else contextlib.nullcontext()
            with loop_cm:
              for b in range(BPC):
                # ---------------- loads ----------------
                Cb = sb.tile([128, LC], f32r, tag="Cb", bufs=3)
                nc.sync.dma_start(out=Cb[:], in_=Cd.ap()[b, :, :])
                Qb = sbs.tile([128, LQ], f32r, tag="Qb")
                nc.sync.dma_start(out=Qb[:], in_=Qd.ap()[b, :, :])
                nQm = sbs.tile([128, 2], f32, tag="nQm")
                nc.sync.dma_start(out=nQm[:], in_=nQd.ap()[b, :, :])
                Cmc = sbs.tile([128, 16], f32r, tag="Cmc")
                nc.sync.dma_start(out=Cmc[:], in_=Cmd.ap()[b, :, :])

                # ---------------- small prep ----------------
                QbW = sbs.tile([128, LQ], f32r, tag="QbW")
                nc.vector.tensor_scalar_mul(out=QbW[:], in0=v32(Qb[:]), scalar1=wmlu[:])
                Qw = sbs.tile([128, LQ], f32r, tag="Qw")
                nc.vector.tensor_scalar(
                    out=Qw[:], in0=v32(Qb[:]), scalar1=wmlu[:], scalar2=wc[:],
                    op0=OP.mult, op1=OP.add,
                )

                # sub1[q] = sum_d Q[d,q] * w4Q[d]  -> [q, 1] per q-chunk
                ps_sub1 = pssm.tile([128, 2], f32, tag="sm")
                for qj in range(2):
                    nc.tensor.matmul(
                        ps_sub1[:, qj : qj + 1],
                        lhsT=v32(Qb[:, 128 * qj : 128 * (qj + 1)]),
                        rhs=wq[:],
                        start=True, stop=True,
                    )
                biasQ = sbs.tile([128, 2], f32, tag="biasQ")
                nc.vector.tensor_add(out=biasQ[:], in0=nQm[:], in1=ps_sub1[:])

                # ---------------- S1 side: N1t [q, c] ----------------
                N1t = []
                for qj in range(2):
                    n1 = sb.tile([128, LC], f32r, tag=f"N1t{qj}")
                    for h in range(2):
                        ps = psbig.tile([128, 1024], f32, tag="bigmm")
                        for n5 in range(2):
                            c0 = 1024 * h + 512 * n5
                            nc.tensor.matmul(
                                ps[:, 512 * n5 : 512 * (n5 + 1)],
                                lhsT=QbW[:, 128 * qj : 128 * (qj + 1)],
                                rhs=Cb[:, c0 : c0 + 512],
                                start=True, stop=True,
                            )
                        nc.scalar.activation(
                            out=n1[:, 1024 * h : 1024 * (h + 1)],
                            in_=ps[:],
                            func=AF.Exp,
                            bias=biasQ[:, qj : qj + 1],
                            scale=1.0,
                        )
                    N1t.append(n1)

                # Qt [q, d] (2 chunks side by side)
                ps_qt = pssm.tile([128, 256], f32r, tag="sm")
                for qj in range(2):
                    nc.tensor.transpose(
                        ps_qt[:, 128 * qj : 128 * (qj + 1)],
                        in_=Qb[:, 128 * qj : 128 * (qj + 1)],
                        identity=ident[:],
                    )
                QtS = sbs.tile([128, 256], f32r, tag="QtS")
                nc.scalar.copy(out=QtS[:], in_=v32(ps_qt[:]))

                # ---------------- CT (transposed, c-masked C) ----------------
                CTm = []
                for g in range(4):
                    ps_ct = pssm.tile([128, 512], f32r, tag="sm")
                    for k in range(4):
                        j = 4 * g + k
                        nc.tensor.transpose(
                            ps_ct[:, 128 * k : 128 * (k + 1)],
                            in_=Cb[:, 128 * j : 128 * (j + 1)],
                            identity=ident[:],
                        )
                    ctm = sb.tile([128, 512], f32r, tag=f"CTm{g}")
                    if g % 2 == 0:
                        nc.scalar.copy(out=ctm[:], in_=v32(ps_ct[:]))
                    else:
                        nc.vector.tensor_copy(ctm[:], v32(ps_ct[:]))
                    for k in range(4):
                        j = 4 * g + k
                        nc.vector.tensor_scalar_mul(
                            out=ctm[:, 128 * k : 128 * (k + 1)],
                            in0=v32(ctm[:, 128 * k : 128 * (k + 1)]),
                            scalar1=v32(Cmc[:, j : j + 1]),
                        )
                    CTm.append(ctm)

                # ---------------- S2 side: N2 [c, q] ----------------
                N2 = []
                for s in range(2):
                    n2 = sb.tile([128, 8, 256], f32r, tag=f"N2{s}")
                    for h in range(2):
                        ps = psbig.tile([128, 1024], f32, tag="bigmm")
                        for k in range(4):
                            j = 8 * s + 4 * h + k
                            nc.tensor.matmul(
                                ps[:, 256 * k : 256 * (k + 1)],
                                lhsT=Cb[:, 128 * j : 128 * (j + 1)],
                                rhs=Qw[:],
                                start=True, stop=True,
                            )
                        nc.scalar.activation(
                            out=n2[:, 4 * h : 4 * (h + 1), :],
                            in_=ps[:],
                            func=AF.Exp,
                        )
                    N2.append(n2)

                # rs[c] broadcast over partitions, then 1/rs
                RBr = sb.tile([128, LC], f32, tag="RBr")
                for h in range(2):
                    ps = psbig.tile([128, 1024], f32, tag="bigmm")
                    for n5 in range(2):
                        c0 = 1024 * h + 512 * n5
                        for qj in range(2):
                            nc.tensor.matmul(
                                ps[:, 512 * n5 : 512 * (n5 + 1)],
                                lhsT=onesm[:],
                                rhs=N1t[qj][:, c0 : c0 + 512],
                                start=(qj == 0), stop=(qj == 1),
                            )
                    nc.vector.reciprocal_approx_fast(
                        out=RBr[:, 1024 * h : 1024 * (h + 1)], in_=ps[:]
                    )

                # cs[q] = sum_c Cm[c] * N2[c, q]  -> [1, 256] psum
                ps_cs = pssm.tile([1, 256], f32, tag="sm")
                for j in range(16):
                    s, jj = divmod(j, 8)
                    nc.tensor.matmul(
                        ps_cs[:],
                        lhsT=Cmc[:, j : j + 1],
                        rhs=N2[s][:, jj, :],
                        start=(j == 0), stop=(j == 15),
                    )
                # transpose cs [1,256] -> [256,1] via PE on a zero-padded tile
                nc.scalar.copy(out=csx[0:1, :], in_=ps_cs[:])
                rcs = sbs.tile([128, 2], f32, tag="rcs")
                for qj in range(2):
                    ps_t = pssm.tile([128, 128], f32, tag="sm")
                    nc.tensor.transpose(
                        ps_t[:],
                        in_=csx[:, 128 * qj : 128 * (qj + 1)],
                        identity=ident32[:],
                    )
                    nc.vector.reciprocal(out=rcs[:, qj : qj + 1], in_=ps_t[:, 0:1])

                # ---------------- V = S2^T @ Ct  [q, d] ----------------
                ps_vt = pssm.tile([128, 256], f32, tag="sm")
                for j in range(16):
                    s, jj = divmod(j, 8)
                    g, k = divmod(j, 4)
                    nc.tensor.matmul(
                        ps_vt[:],
                        lhsT=CTm[g][:, 128 * k : 128 * (k + 1)],
                        rhs=N2[s][:, jj, :],
                        start=(j == 0), stop=(j == 15),
                    )
                VtS = sbs.tile([128, 256], f32r, tag="VtS")
                nc.vector.tensor_copy(VtS[:], ps_vt[:])
                ps_v = pssm.tile([128, 256], f32r, tag="sm")
                for qj in range(2):
                    nc.tensor.transpose(
                        ps_v[:, 128 * qj : 128 * (qj + 1)],
                        in_=VtS[:, 128 * qj : 128 * (qj + 1)],
                        identity=ident[:],
                    )
                Vs = sbs.tile([128, 256], f32r, tag="Vs")
                for qj in range(2):
                    nc.scalar.activation(
                        out=Vs[:, 128 * qj : 128 * (qj + 1)],
                        in_=v32(ps_v[:, 128 * qj : 128 * (qj + 1)]),
                        func=AF.Copy,
                        scale=rcs[:, qj : qj + 1],
                    )

                # ---------------- outputs ----------------
                # (row-block 0 of the final output is C itself; it is
                # assembled host-side, never shipped through the device)

                # A^T and Bt^T raw matmuls + normalization + C products
                o2 = sb.tile([128, LC], f32, tag="o2")
                o4a = sb.tile([128, LC], f32, tag="o4a")
                for h in range(2):
                    ps_at = psbig.tile([128, 1024], f32, tag="bigmm")
                    for n5 in range(2):
                        c0 = 1024 * h + 512 * n5
                        for qj in range(2):
                            nc.tensor.matmul(
                                ps_at[:, 512 * n5 : 512 * (n5 + 1)],
                                lhsT=QtS[:, 128 * qj : 128 * (qj + 1)],
                                rhs=N1t[qj][:, c0 : c0 + 512],
                                start=(qj == 0), stop=(qj == 1),
                            )
                    nc.vector.tensor_mul(
                        out=o2[:, 1024 * h : 1024 * (h + 1)],
                        in0=ps_at[:],
                        in1=RBr[:, 1024 * h : 1024 * (h + 1)],
                    )
                for h in range(2):
                    ps_bt = psbig.tile([128, 1024], f32, tag="bigmm")
                    for n5 in range(2):
                        c0 = 1024 * h + 512 * n5
                        for qj in range(2):
                            nc.tensor.matmul(
                                ps_bt[:, 512 * n5 : 512 * (n5 + 1)],
                                lhsT=Vs[:, 128 * qj : 128 * (qj + 1)],
                                rhs=N1t[qj][:, c0 : c0 + 512],
                                start=(qj == 0), stop=(qj == 1),
                            )
                    nc.vector.tensor_mul(
                        out=o4a[:, 1024 * h : 1024 * (h + 1)],
                        in0=ps_bt[:],
                        in1=RBr[:, 1024 * h : 1024 * (h + 1)],
                    )
                nc.sync.dma_start(out=outd.ap()[b, 0:128, :], in_=o2[:])

                o3 = sb.tile([128, LC], f32, tag="o3", bufs=1)
                _e34 = getattr(nc, O34_ENG)
                _e34.tensor_mul(out=o3[:], in0=o2[:], in1=v32(Cb[:]))
                nc.sync.dma_start(out=outd.ap()[b, 128:256, :], in_=o3[:])

                o4 = sb.tile([128, LC], f32, tag="o4", bufs=1)
                _e34.tensor_mul(out=o4[:], in0=o4a[:], in1=v32(Cb[:]))
                nc.sync.dma_start(out=outd.ap()[b, 256:384, :], in_=o4[:])

    nc.compile()
    return nc


def _get_program(repeat=1):
    key = f"nc{repeat}"
    if key not in _CACHE:
        _CACHE[key] = _build_program(repeat)
    return _CACHE[key]


def _shard_inputs(C, Q, Cmask, Qmask, w4C, w4Q, w4mlu):
    C = np.ascontiguousarray(C, dtype=np.float32)
    Q = np.ascontiguousarray(Q, dtype=np.float32)
    negQm = (NEG_BIG * (1.0 - Qmask.astype(np.float32))).astype(np.float32)
    # [B, LQ] -> [B, 2, 128] -> [B, 128, 2]
    negQm = np.ascontiguousarray(negQm.reshape(B_FULL, 2, 128).transpose(0, 2, 1))
    Cmf = Cmask.astype(np.float32).reshape(B_FULL, 16, 128).transpose(0, 2, 1)
    Cmf = np.ascontiguousarray(Cmf)
    wmlu = np.ascontiguousarray(np.asarray(w4mlu, dtype=np.float32).reshape(D, 1))
    wc = np.ascontiguousarray(np.asarray(w4C, dtype=np.float32).reshape(D, 1))
    wq = np.ascontiguousarray(np.asarray(w4Q, dtype=np.float32).reshape(D, 1))
    in_maps = []
    for i in range(N_CORES):
        sl = slice(BPC * i, BPC * (i + 1))
        in_maps.append(
            {
                "C": C[sl],
                "Q": Q[sl],
                "negQm": negQm[sl],
                "Cmf": Cmf[sl],
                "wmlu": wmlu,
                "wc": wc,
                "wq": wq,
            }
        )
    return in_maps


def kernel(C, Q, Cmask, Qmask, w4C, w4Q, w4mlu, bias):
    # bias is a scalar added to every logit; it cancels in both softmaxes and
    # never reaches the output, so it is accepted and ignored.
    from concourse.bass_utils import run_bass_kernel_spmd

    nc = _get_program()
    in_maps = _shard_inputs(C, Q, Cmask, Qmask, w4C, w4Q, w4mlu)
    res = run_bass_kernel_spmd(nc, in_maps, list(range(N_CORES)))
    dev = np.concatenate([res.results[i]["out"] for i in range(N_CORES)], axis=0)
    out = np.empty((B_FULL, 4 * D, LC), dtype=np.float32)
    out[:, 0:D, :] = np.asarray(C, dtype=np.float32)
    out[:, D:, :] = dev
    return out



# revision 3
# speedup vs baseline: 4.2296x; 4.2296x over previous
"""CQAttention (QANet context-query attention) on 8 Trainium2 NeuronCores.

Full inputs in, full output out. Data-parallel over batch B=32 -> 4 batches
per core. See _build_program() for the per-core Bass/Tile program.

Math notes (vs the jax reference):
  - `bias` and the cross-terms sub0/sub1 that are constant along a softmax
    axis drop out of that softmax; sub1 enters S1's logits as a per-q bias,
    sub0 enters S2's logits via folding w4C into the rhs of the S2 matmul.
  - All matmul operands, attention matrices and outputs are bf16 (PSUM
    accumulation stays f32); host pre-computes the transposed (and
    Cmask-pre-masked) variants of C and Q plus the weighted Q tensors, so
    the device performs no input transposes or mask multiplies.
  - S1 = softmax_q: N1t computed transposed [q, c]; row-sum rs[c] over q via
    an all-ones lhsT matmul (arrives pre-broadcast over 128 partitions);
    1/rs applied to the final A^T/Bt^T.
  - S2 = softmax_c: N2 computed [c, q] unmasked; the c-mask enters via the
    host-pre-masked Ct (V matmul) and the Cmf lhsT column (cs matmul).
  - The per-core batch loop is software-pipelined: loads/logits/exps for
    batch b overlap reductions/V/A/B/outputs for batch b-1.
"""

import os
import sys

for _p in ("/opt/trn_rl_repo", "/root/.axon_site/_ro/trn_rl_repo"):
    if os.path.isdir(_p) and _p not in sys.path:
        sys.path.insert(0, _p)

import numpy as np

N_CORES = 8
B_FULL = 32
BPC = B_FULL // N_CORES  # batches per core
D = 128
LC = 2048
LQ = 256
NEG_BIG = -30000.0

_CACHE = {}


def _build_program(repeat=1):
    import concourse.mybir as mybir
    import concourse.tile as tile
    from concourse import bacc
    from concourse.masks import make_identity

    f32 = mybir.dt.float32
    bf16 = mybir.dt.bfloat16
    AF = mybir.ActivationFunctionType

    nc = bacc.Bacc("TRN2", target_bir_lowering=False, debug=False)

    Cd = nc.dram_tensor("C", [BPC, D, LC], bf16, kind="ExternalInput")
    Ctd = nc.dram_tensor("Ct", [BPC, D, LC], bf16, kind="ExternalInput")
    Qpd = nc.dram_tensor("Qpack", [BPC, D, 3 * LQ + 16], bf16, kind="ExternalInput")
    bpd = nc.dram_tensor("bpack", [BPC, D, 2], f32, kind="ExternalInput")
    outd = nc.dram_tensor("out", [BPC, 3 * D, LC], bf16, kind="ExternalOutput")

    with tile.TileContext(nc) as tc:
        with (
            tc.tile_pool(name="const", bufs=1) as constp,
            tc.tile_pool(name="big", bufs=2) as sb,
            tc.tile_pool(name="small", bufs=2) as sbs,
            tc.tile_pool(name="psbig", bufs=3, space="PSUM") as psbig,
            tc.tile_pool(name="pssm", bufs=2, space="PSUM") as pssm,
        ):
            ident32 = constp.tile([128, 128], f32)
            make_identity(nc, ident32[:])
            identb = constp.tile([128, 128], bf16)
            nc.vector.tensor_copy(identb[:], ident32[:])
            onesb = constp.tile([128, 128], bf16)
            nc.vector.memset(onesb[:], 1.0)
            csx = constp.tile([128, 256], f32)
            nc.vector.memset(csx[:], 0.0)

            # per-batch state handed from stage1 to stage2
            state = {}

            def stage1(b):
                # loads + both logit GEMMs + exps
                Cb = sb.tile([128, LC], bf16, tag="Cb", bufs=3)
                nc.sync.dma_start(out=Cb[:], in_=Cd.ap()[b, :, :])
                Ct = sb.tile([128, LC], bf16, tag="Ct")
                nc.sync.dma_start(out=Ct[:], in_=Ctd.ap()[b, :, :])
                Qp = sbs.tile([128, 3 * LQ + 16], bf16, tag="Qp")
                nc.sync.dma_start(out=Qp[:], in_=Qpd.ap()[b, :, :])
                bp = sbs.tile([128, 2], f32, tag="bp")
                nc.sync.dma_start(out=bp[:], in_=bpd.ap()[b, :, :])
                QbW = Qp[:, 0:256]
                Qw = Qp[:, 256:512]
                Qt = Qp[:, 512:768]
                Cmf = Qp[:, 768:784]
                biasQ = bp[:, 0:2]

                # S1 side: N1t [q, c] = exp(sub2^T + sub1 + qmask)
                N1t = []
                for qj in range(2):
                    n1 = sb.tile([128, LC], bf16, tag=f"N1t{qj}")
                    for h in range(2):
                        ps = psbig.tile([128, 1024], f32, tag="bigmm")
                        for n5 in range(2):
                            c0 = 1024 * h + 512 * n5
                            nc.tensor.matmul(
                                ps[:, 512 * n5 : 512 * (n5 + 1)],
                                lhsT=QbW[:, 128 * qj : 128 * (qj + 1)],
                                rhs=Cb[:, c0 : c0 + 512],
                                start=True, stop=True,
                            )
                        nc.scalar.activation(
                            out=n1[:, 1024 * h : 1024 * (h + 1)],
                            in_=ps[:],
                            func=AF.Exp,
                            bias=biasQ[:, qj : qj + 1],
                            scale=1.0,
                        )
                    N1t.append(n1)

                # S2 side: N2 [c, q] = exp(sub2 + sub0), unmasked
                N2 = []
                for s in range(2):
                    n2 = sb.tile([128, 8, 256], bf16, tag=f"N2{s}")
                    for h in range(2):
                        ps = psbig.tile([128, 1024], f32, tag="bigmm")
                        for k in range(4):
                            j = 8 * s + 4 * h + k
                            nc.tensor.matmul(
                                ps[:, 256 * k : 256 * (k + 1)],
                                lhsT=Cb[:, 128 * j : 128 * (j + 1)],
                                rhs=Qw[:],
                                start=True, stop=True,
                            )
                        nc.scalar.activation(
                            out=n2[:, 4 * h : 4 * (h + 1), :],
                            in_=ps[:],
                            func=AF.Exp,
                        )
                    N2.append(n2)
                state[b] = (Cb, Ct, Qt, Cmf, N1t, N2)

            def stage2(b):
                Cb, Ct, Qt, Cmf, N1t, N2 = state.pop(b)

                # rs[c] broadcast over partitions, then 1/rs
                RBr = sb.tile([128, LC], f32, tag="RBr")
                for h in range(2):
                    ps = psbig.tile([128, 1024], f32, tag="bigmm")
                    for n5 in range(2):
                        c0 = 1024 * h + 512 * n5
                        for qj in range(2):
                            nc.tensor.matmul(
                                ps[:, 512 * n5 : 512 * (n5 + 1)],
                                lhsT=onesb[:],
                                rhs=N1t[qj][:, c0 : c0 + 512],
                                start=(qj == 0), stop=(qj == 1),
                            )
                    nc.vector.reciprocal_approx_fast(
                        out=RBr[:, 1024 * h : 1024 * (h + 1)], in_=ps[:]
                    )

                # cs[q] = sum_c Cm[c] * N2[c, q]  -> [1, 256] psum
                ps_cs = pssm.tile([1, 256], f32, tag="sm")
                for j in range(16):
                    s, jj = divmod(j, 8)
                    nc.tensor.matmul(
                        ps_cs[:],
                        lhsT=Cmf[:, j : j + 1],
                        rhs=N2[s][:, jj, :],
                        start=(j == 0), stop=(j == 15),
                    )
                # transpose cs [1,256] -> [256,1] via PE on a zero-padded tile
                nc.scalar.copy(out=csx[0:1, :], in_=ps_cs[:])
                rcs = sbs.tile([128, 2], f32, tag="rcs")
                for qj in range(2):
                    ps_t = pssm.tile([128, 128], f32, tag="sm")
                    nc.tensor.transpose(
                        ps_t[:],
                        in_=csx[:, 128 * qj : 128 * (qj + 1)],
                        identity=ident32[:],
                    )
                    nc.vector.reciprocal(out=rcs[:, qj : qj + 1], in_=ps_t[:, 0:1])

                # V = S2^T @ Ct  [q, d] (Ct arrives pre-masked from host)
                ps_vt = pssm.tile([128, 256], f32, tag="sm")
                for j in range(16):
                    s, jj = divmod(j, 8)
                    nc.tensor.matmul(
                        ps_vt[:],
                        lhsT=Ct[:, 128 * j : 128 * (j + 1)],
                        rhs=N2[s][:, jj, :],
                        start=(j == 0), stop=(j == 15),
                    )
                VtS = sbs.tile([128, 256], bf16, tag="VtS")
                nc.vector.tensor_copy(VtS[:], ps_vt[:])
                ps_v = pssm.tile([128, 256], bf16, tag="sm")
                for qj in range(2):
                    nc.tensor.transpose(
                        ps_v[:, 128 * qj : 128 * (qj + 1)],
                        in_=VtS[:, 128 * qj : 128 * (qj + 1)],
                        identity=identb[:],
                    )
                Vs = sbs.tile([128, 256], bf16, tag="Vs")
                for qj in range(2):
                    nc.scalar.activation(
                        out=Vs[:, 128 * qj : 128 * (qj + 1)],
                        in_=ps_v[:, 128 * qj : 128 * (qj + 1)],
                        func=AF.Copy,
                        scale=rcs[:, qj : qj + 1],
                    )

                # outputs (row-block 0 of the final output is C itself,
                # assembled host-side)
                o2 = sb.tile([128, LC], bf16, tag="o2")
                for h in range(2):
                    ps_at = psbig.tile([128, 1024], f32, tag="bigmm")
                    for n5 in range(2):
                        c0 = 1024 * h + 512 * n5
                        for qj in range(2):
                            nc.tensor.matmul(
                                ps_at[:, 512 * n5 : 512 * (n5 + 1)],
                                lhsT=Qt[:, 128 * qj : 128 * (qj + 1)],
                                rhs=N1t[qj][:, c0 : c0 + 512],
                                start=(qj == 0), stop=(qj == 1),
                            )
                    nc.vector.tensor_mul(
                        out=o2[:, 1024 * h : 1024 * (h + 1)],
                        in0=ps_at[:],
                        in1=RBr[:, 1024 * h : 1024 * (h + 1)],
                    )
                o4a = sb.tile([128, LC], bf16, tag="o4a")
                for h in range(2):
                    ps_bt = psbig.tile([128, 1024], f32, tag="bigmm")
                    for n5 in range(2):
                        c0 = 1024 * h + 512 * n5
                        for qj in range(2):
                            nc.tensor.matmul(
                                ps_bt[:, 512 * n5 : 512 * (n5 + 1)],
                                lhsT=Vs[:, 128 * qj : 128 * (qj + 1)],
                                rhs=N1t[qj][:, c0 : c0 + 512],
                                start=(qj == 0), stop=(qj == 1),
                            )
                    nc.vector.tensor_mul(
                        out=o4a[:, 1024 * h : 1024 * (h + 1)],
                        in0=ps_bt[:],
                        in1=RBr[:, 1024 * h : 1024 * (h + 1)],
                    )
                nc.sync.dma_start(out=outd.ap()[b, 0:128, :], in_=o2[:])

                o3 = sb.tile([128, LC], bf16, tag="o3")
                nc.vector.tensor_mul(out=o3[:], in0=o2[:], in1=Cb[:])
                nc.sync.dma_start(out=outd.ap()[b, 128:256, :], in_=o3[:])

                o4 = sb.tile([128, LC], bf16, tag="o4")
                nc.vector.tensor_mul(out=o4[:], in0=o4a[:], in1=Cb[:])
                nc.sync.dma_start(out=outd.ap()[b, 256:384, :], in_=o4[:])

            import contextlib
            loop_cm = tc.For_i(0, repeat) if repeat > 1 else contextlib.nullcontext()
            with loop_cm:
                for b in range(BPC + 1):
                    if b < BPC:
                        stage1(b)
                    if b > 0:
                        stage2(b - 1)

    nc.compile()
    return nc


def _get_program(repeat=1):
    key = f"nc{repeat}"
    if key not in _CACHE:
        _CACHE[key] = _build_program(repeat)
    return _CACHE[key]


def _shard_inputs(C, Q, Cmask, Qmask, w4C, w4Q, w4mlu):
    import ml_dtypes

    bf16 = ml_dtypes.bfloat16
    C = np.asarray(C, dtype=np.float32)
    Q = np.asarray(Q, dtype=np.float32)
    Cmaskf = np.asarray(Cmask, dtype=np.float32)
    wmlu = np.asarray(w4mlu, dtype=np.float32).reshape(D)
    wc = np.asarray(w4C, dtype=np.float32).reshape(D)
    wq = np.asarray(w4Q, dtype=np.float32).reshape(D)

    Cb = C.astype(bf16)
    # Ct[b][cc, g*128 + dd] = Cmask[b, g*128+cc] * C[b, dd, g*128 + cc]
    Ctm = C * Cmaskf[:, None, :]
    Ct = np.ascontiguousarray(
        Ctm.reshape(B_FULL, D, 16, 128).transpose(0, 3, 2, 1)
    ).reshape(B_FULL, D, LC).astype(bf16)
    QbW = (Q * wmlu[None, :, None]).astype(bf16)
    Qw = (Q * wmlu[None, :, None] + wc[None, :, None]).astype(bf16)
    Qt = np.ascontiguousarray(
        Q.reshape(B_FULL, D, 2, 128).transpose(0, 3, 2, 1)
    ).reshape(B_FULL, D, LQ).astype(bf16)
    Cmf = np.ascontiguousarray(
        Cmaskf.reshape(B_FULL, 16, 128).transpose(0, 2, 1)
    ).astype(bf16)
    Qpack = np.ascontiguousarray(np.concatenate([QbW, Qw, Qt, Cmf], axis=2))

    sub1 = np.einsum("bdq,d->bq", Q, wq)  # [B, LQ]
    biasQ = sub1 + NEG_BIG * (1.0 - Qmask.astype(np.float32))
    bpack = np.ascontiguousarray(
        biasQ.reshape(B_FULL, 2, 128).transpose(0, 2, 1)
    ).astype(np.float32)

    in_maps = []
    for i in range(N_CORES):
        sl = slice(BPC * i, BPC * (i + 1))
        in_maps.append(
            {
                "C": Cb[sl],
                "Ct": Ct[sl],
                "Qpack": Qpack[sl],
                "bpack": bpack[sl],
            }
        )
    return in_maps


def kernel(C, Q, Cmask, Qmask, w4C, w4Q, w4mlu, bias):
    # bias is a scalar added to every logit; it cancels in both softmaxes and
    # never reaches the output, so it is accepted and ignored.
    from concourse.bass_utils import run_bass_kernel_spmd

    nc = _get_program()
    in_maps = _shard_inputs(C, Q, Cmask, Qmask, w4C, w4Q, w4mlu)
    res = run_bass_kernel_spmd(nc, in_maps, list(range(N_CORES)))
    dev = np.concatenate([res.results[i]["out"] for i in range(N_CORES)], axis=0)
    out = np.empty((B_FULL, 4 * D, LC), dtype=np.float32)
    out[:, 0:D, :] = np.asarray(C, dtype=np.float32)
    out[:, D:, :] = dev.astype(np.float32)
    return out
